# revision 23
# baseline (speedup 1.0000x reference)
"""DCT sequence-compression kernel for TRN2 (nn_CompressedModel).

For x [B=64, T=1024, D=768] fp32 computes (matching the reference):
  x_dct = (C_T @ x)[:, :k, :]          k = 922
  x_rec = C_k^T @ x_dct
returning (x_rec, x_dct).

Mirror symmetries cut the tensor-engine streaming ~3.4x vs the naive
pair of matmuls (PE streams ~0.52 ns/col sustained, both bf16 and
fp32r, so fewer columns is the only PE lever):

1) Input fold (host): C[k, T-1-t] = (-1)^k C[k, t], so with
   e = x[:512] + rev(x[512:]), o = x[:512] - rev(x[512:]) the even dct
   rows contract only e and the odd rows only o (512-long).

2) Second input fold (host): C[k, H-1-t] = +/- C[k, t] for k = 0,2
   mod 4, so with ee = e[:256] + rev(e[256:]), eo = e[:256] - rev(...),
   dct rows 4i contract only ee and rows 4i+2 only eo (256-long).
   (Odd rows don't fold again — the quarter reflection maps them onto
   a DST basis.)

3) Output fold: P = C_k^T C_trunc satisfies P[K-1-n, t] = P[n, T-1-t],
   making the folded reconstruction weights column-(anti)symmetric:
   wce[:, K-1-n] = wce[:, n], wco[:, K-1-n] = -wco[:, n]. Hence
     s = wce[:, :461]^T e     d = wco[:, :461]^T o
   give rec[n] = s[n] + d[n], rec[K-1-n] = s[n] - d[n] for n < 461 —
   half the reconstruction streaming; the combine is two vector ops.

All matmuls run in bf16 (same PE rate as fp32r here, half the HBM
traffic); accumulation stays fp32 in PSUM, outputs are written bf16
and upcast on host. End-to-end rel err ~3e-3 vs the 2e-2 gate.
Pure data parallel over B across 8 cores.
"""

import os

import ml_dtypes
import numpy as np

# The trimmed axon environment has no NTFF profile hook; make sure
# run_bass_kernel_spmd never tries the trace path.
os.environ["BASS_NEVER_TRACE"] = "1"

import concourse.bass as bass  # noqa: F401
import concourse.mybir as mybir
import concourse.tile as tile
from concourse import bacc
from concourse.bass_utils import run_bass_kernel_spmd

B, T, D = 64, 1024, 768
K = 922              # ceil(0.9 * 1024)
KPAD = 924           # dct rows padded to a multiple of 4 on device
KH = K // 2          # 461: folded output rows
H = T // 2           # 512: e/o contraction length
Q = T // 4           # 256: ee/eo contraction length
NEE = 231            # dct rows k % 4 == 0
NEO = 230            # dct rows k % 4 == 2
N_CORES = 8
BPC = B // N_CORES   # batches per core
P = 128
CC = H // P          # 4 contraction chunks for e/o
CCQ = Q // P         # 2 contraction chunks for ee/eo
N0 = 512             # first free-dim split (PSUM bank)

BF16 = mybir.dt.bfloat16
NPBF16 = ml_dtypes.bfloat16


def _chunks(n, p=P):
    return [(i * p, min(p, n - i * p)) for i in range((n + p - 1) // p)]


R_CHUNKS = _chunks(KH)    # 4 chunks: 128,128,128,77
EE_CHUNKS = _chunks(NEE)  # 2 chunks: 128,103
EO_CHUNKS = _chunks(NEO)  # 2 chunks: 128,102

# experiment knobs (bench-time only; defaults are the shipped config)
REC_RING = os.environ.get("KERNEL_REC_RING", "scalar")   # rec2 dma engine
DCT_RING = os.environ.get("KERNEL_DCT_RING", "sync")     # dct dma engine
# NOTE: Pool/GPSIMD cannot read PSUM on TRN2, so PSUM drains must go to
# the ACT (scalar) or DVE (vector) engines.
ST_ENG = os.environ.get("KERNEL_ST_ENG", "scalar")       # st copy engine
XP_BUFS = int(os.environ.get("KERNEL_XP_BUFS", "3"))
OP_BUFS = int(os.environ.get("KERNEL_OP_BUFS", "6"))
# probe knobs: strip parts of the pipeline to attribute time
NO_OUT_DMA = bool(os.environ.get("KERNEL_NO_OUT_DMA"))
NO_COMPUTE = bool(os.environ.get("KERNEL_NO_COMPUTE"))
NO_IN_DMA = bool(os.environ.get("KERNEL_NO_IN_DMA"))


def _dct_matrix(N: int) -> np.ndarray:
    """Orthonormal DCT-II matrix [N, N] in float64."""
    n = np.arange(N, dtype=np.float64)
    C = np.cos(np.pi * (2.0 * n[None, :] + 1.0) * n[:, None] / (2.0 * N))
    s = np.full(N, np.sqrt(2.0 / N))
    s[0] = np.sqrt(1.0 / N)
    return s[:, None] * C


def _build_weights():
    C_T = _dct_matrix(T)
    C_k = _dct_matrix(K)
    W2 = (C_k.T @ C_T[:K, :]).T            # [T, K]: x -> x_rec columns
    W2r = W2[::-1, :]
    wce = (W2[:H, :] + W2r[:H, :]) / 2.0   # [H, K] vs e; cols mirror-sym
    wco = (W2[:H, :] - W2r[:H, :]) / 2.0   # [H, K] vs o; cols mirror-anti
    ws = wce[:, :KH]                                    # [H, 461] s weights
    wo = np.concatenate([C_T[1:K:2, :H].T, wco[:, :KH]], axis=1)  # [H, 922]
    wee = C_T[0:K:4, :Q].T                              # [Q, 231]
    weo = C_T[2:K:4, :Q].T                              # [Q, 230]
    return (ws.astype(NPBF16), wo.astype(NPBF16),
            wee.astype(NPBF16), weo.astype(NPBF16))


def _build_bass(loop_repeat: int = 1):
    """loop_repeat>1 wraps the program in a hardware For_i loop (same
    outputs each trip) — used by test.py for slope-based HW timing."""
    f32 = mybir.dt.float32
    nc = bacc.Bacc("TRN2", target_bir_lowering=False, debug=False,
                   num_devices=N_CORES)
    e_in = nc.dram_tensor("e", [BPC, H, D], BF16, kind="ExternalInput").ap()
    o_in = nc.dram_tensor("o", [BPC, H, D], BF16, kind="ExternalInput").ap()
    # eeo = concat(ee, eo) along tokens: cc 0..1 = ee, cc 2..3 = eo
    eeo_in = nc.dram_tensor("eeo", [BPC, H, D], BF16,
                            kind="ExternalInput").ap()
    ws_in = nc.dram_tensor("ws", [H, KH], BF16, kind="ExternalInput").ap()
    wo_in = nc.dram_tensor("wo", [H, 2 * KH], BF16,
                           kind="ExternalInput").ap()
    wee_in = nc.dram_tensor("wee", [Q, NEE], BF16,
                            kind="ExternalInput").ap()
    weo_in = nc.dram_tensor("weo", [Q, NEO], BF16,
                            kind="ExternalInput").ap()
    dct = nc.dram_tensor("dct", [BPC, KPAD, D], BF16,
                         kind="ExternalOutput").ap()
    # rec2[:, :, 0] = rec rows 0..460; rec2[:, :, 1, n] = rec row 921-n
    rec2 = nc.dram_tensor("rec2", [BPC, KH, 2, D], BF16,
                          kind="ExternalOutput").ap()

    # dct row views: rows 4i (ee), 4i+2 (eo), 2j+1 (odd)
    dct_q = dct.rearrange("b (i four) d -> b i four d", four=4)
    dct_o = dct.rearrange("b (j two) d -> b j two d", two=2)
    e_r = e_in.rearrange("b (c p) d -> b p c d", p=P)
    o_r = o_in.rearrange("b (c p) d -> b p c d", p=P)
    eeo_r = eeo_in.rearrange("b (c p) d -> b p c d", p=P)
    ws_r = ws_in.rearrange("(c p) j -> p c j", p=P)
    wo_r = wo_in.rearrange("(c p) j -> p c j", p=P)
    wee_r = wee_in.rearrange("(c p) j -> p c j", p=P)
    weo_r = weo_in.rearrange("(c p) j -> p c j", p=P)

    with tile.TileContext(nc) as tc:
        with (
            tc.tile_pool(name="wp", bufs=1) as wp,
            tc.tile_pool(name="xp", bufs=XP_BUFS) as xp,
            tc.tile_pool(name="op", bufs=OP_BUFS) as op,
            tc.tile_pool(name="pp", bufs=4, space="PSUM") as pp,
        ):
            wst = wp.tile([P, CC, KH], BF16)
            wot = wp.tile([P, CC, 2 * KH], BF16)
            weet = wp.tile([P, CCQ, NEE], BF16)
            weot = wp.tile([P, CCQ, NEO], BF16)
            # Weights stream on the ACT HWDGE ring in consumption order;
            # inputs/outputs use the SP ring.
            nc.scalar.dma_start(weet[:], wee_r[:])
            nc.scalar.dma_start(weot[:], weo_r[:])
            for (c0, sz) in R_CHUNKS:
                nc.scalar.dma_start(wot[:, :, c0:c0 + sz],
                                    wo_r[:, :, c0:c0 + sz])
            for (c0, sz) in R_CHUNKS:
                nc.scalar.dma_start(wst[:, :, c0:c0 + sz],
                                    ws_r[:, :, c0:c0 + sz])
                nc.scalar.dma_start(wot[:, :, KH + c0:KH + c0 + sz],
                                    wo_r[:, :, KH + c0:KH + c0 + sz])

            if NO_COMPUTE:
                soc = wp.tile([P, D], BF16)
                src = wp.tile([P, 2, D], BF16)
                nc.sync.dma_start(soc[:], wo_r[:, 0, 0:D])
                nc.sync.dma_start(src[:], wo_r[:, 0:2, 0:D])
            if NO_IN_DMA:
                etc = wp.tile([P, CC, D], BF16)
                otc = wp.tile([P, CC, D], BF16)
                eeoc = wp.tile([P, CC, D], BF16)
                nc.sync.dma_start(etc[:], e_r[0])
                nc.sync.dma_start(otc[:], o_r[0])
                nc.sync.dma_start(eeoc[:], eeo_r[0])

            def mm_group(pt, wtile, c0, rhs, sz, ccs, rhs_cc0=0):
                """Accumulate sum_cc w[cc].T @ rhs[rhs_cc0+cc] into pt."""
                for i, cc in enumerate(ccs):
                    st, sp = (i == 0), (i == len(ccs) - 1)
                    nc.tensor.matmul(
                        pt[:sz, 0:N0], wtile[:, cc, c0:c0 + sz],
                        rhs[:, rhs_cc0 + cc, 0:N0], start=st, stop=sp)
                    nc.tensor.matmul(
                        pt[:sz, N0:D], wtile[:, cc, c0:c0 + sz],
                        rhs[:, rhs_cc0 + cc, N0:D], start=st, stop=sp)

            def dct_emit(wtile, r0, sz, rhs, ccs, dest, rhs_cc0=0):
                """matmul group -> ACT copy -> dma to a dct row view."""
                so = (soc if NO_COMPUTE
                      else op.tile([P, D], BF16, tag="so"))
                if not NO_COMPUTE:
                    pt = pp.tile([P, D], f32, tag="pt")
                    mm_group(pt, wtile, r0, rhs, sz, ccs, rhs_cc0)
                    nc.scalar.copy(so[:sz, :], pt[:sz, :])
                if not NO_OUT_DMA:
                    getattr(nc, DCT_RING).dma_start(dest, so[:sz, :])

            def body():
                for b in range(BPC):
                    if NO_IN_DMA:
                        et, ot, eeot = etc, otc, eeoc
                    else:
                        et = xp.tile([P, CC, D], BF16, tag="et")
                        ot = xp.tile([P, CC, D], BF16, tag="ot")
                        eeot = xp.tile([P, CC, D], BF16, tag="eeot")
                        nc.sync.dma_start(et[:], e_r[b])
                        nc.sync.dma_start(ot[:], o_r[b])
                        nc.sync.dma_start(eeot[:], eeo_r[b])

                    # dct rows 4i (ee) and 4i+2 (eo): 256-long contraction
                    for (r0, sz) in EE_CHUNKS:
                        dct_emit(weet, r0, sz, eeot, (0, 1),
                                 dct_q[b, r0:r0 + sz, 0, :], rhs_cc0=0)
                    for (r0, sz) in EO_CHUNKS:
                        dct_emit(weot, r0, sz, eeot, (0, 1),
                                 dct_q[b, r0:r0 + sz, 2, :], rhs_cc0=2)

                    for (r0, sz) in R_CHUNKS:
                        # dct odd rows 2j+1 (from o)
                        dct_emit(wot, r0, sz, ot, (0, 1, 2, 3),
                                 dct_o[b, r0:r0 + sz, 1, :])

                        # rec halves: s (from e) and d (from o). The DVE
                        # can read only one PSUM operand per tensor_tensor,
                        # so s is staged through SBUF by the ACT engine.
                        sr = (src if NO_COMPUTE
                              else op.tile([P, 2, D], BF16, tag="sr"))
                        if not NO_COMPUTE:
                            pt_s = pp.tile([P, D], f32, tag="pt")
                            mm_group(pt_s, wst, r0, et, sz, (0, 1, 2, 3))
                            pt_d = pp.tile([P, D], f32, tag="pt")
                            mm_group(pt_d, wot, KH + r0, ot, sz,
                                     (0, 1, 2, 3))
                            st = op.tile([P, D], f32, tag="st")
                            if ST_ENG == "scalar":
                                nc.scalar.copy(st[:sz, :], pt_s[:sz, :])
                            else:
                                getattr(nc, ST_ENG).tensor_copy(
                                    st[:sz, :], pt_s[:sz, :])
                            nc.vector.tensor_add(sr[:sz, 0, :],
                                                 pt_d[:sz, :], st[:sz, :])
                            nc.vector.tensor_sub(sr[:sz, 1, :], st[:sz, :],
                                                 pt_d[:sz, :])
                        if not NO_OUT_DMA:
                            getattr(nc, REC_RING).dma_start(
                                rec2[b, r0:r0 + sz], sr[:sz])

            if loop_repeat > 1:
                with tc.For_i(0, loop_repeat, 1):
                    body()
            else:
                body()
    nc.compile()
    return nc


_CACHE = {}


def _get():
    if "nc" not in _CACHE:
        _CACHE["nc"] = _build_bass()
        _CACHE["w"] = _build_weights()
    return _CACHE["nc"], _CACHE["w"]


def _make_in_maps(x: np.ndarray):
    _, w = _get()
    ws, wo, wee, weo = w
    x = np.asarray(x, dtype=np.float32)
    lo = x[:, :H, :]
    hi = x[:, :H - 1:-1, :]
    ef = lo + hi
    of = lo - hi
    ee = ef[:, :Q, :] + ef[:, :Q - 1:-1, :]
    eo = ef[:, :Q, :] - ef[:, :Q - 1:-1, :]
    e = np.ascontiguousarray(ef, dtype=NPBF16)
    o = np.ascontiguousarray(of, dtype=NPBF16)
    eeo = np.concatenate([ee, eo], axis=1).astype(NPBF16)
    return [
        {"e": e[c * BPC:(c + 1) * BPC], "o": o[c * BPC:(c + 1) * BPC],
         "eeo": eeo[c * BPC:(c + 1) * BPC],
         "ws": ws, "wo": wo, "wee": wee, "weo": weo}
        for c in range(N_CORES)
    ]


def kernel(x: np.ndarray, _results_out=None):
    """x [64, 1024, 768] fp32 -> (x_rec [64, 922, 768], x_dct [64, 922, 768])."""
    nc, _ = _get()
    in_maps = _make_in_maps(x)
    res = run_bass_kernel_spmd(nc, in_maps, core_ids=list(range(N_CORES)))
    if _results_out is not None:
        _results_out.append(res)
    x_dct = np.concatenate(
        [np.asarray(r["dct"])[:, :K, :] for r in res.results], axis=0
    ).astype(np.float32)
    rec_parts = []
    for r in res.results:
        r2 = np.asarray(r["rec2"])                      # [BPC, 461, 2, D]
        lo = r2[:, :, 0, :]
        hi = r2[:, ::-1, 1, :]                          # rec rows 461..921
        rec_parts.append(np.concatenate([lo, hi], axis=1))
    x_rec = np.concatenate(rec_parts, axis=0).astype(np.float32)
    return x_rec, x_dct
